# revision 24
# baseline (speedup 1.0000x reference)
"""Multi-head attention (B=2, S=2048, D=1024, H=16, hd=64) on 8 TRN2 cores.

Sharding: tensor-parallel on heads x data-parallel on batch. Core c handles
batch b=c//4 and heads [4*(c%4), 4*(c%4)+4). Each core computes the fused QKV
projection for its head slice, attention for its 4 heads, and a partial
out-projection (out_w column slice); the host sums the 4 partials per batch
and adds out_b.

fp8 attention core (v2):
- q/k/v are quantized to fp8e4m3 on eviction from the projection psum.
- scores^T via fp8 DoubleRow matmuls (2 MACs/cell/cycle). The K=64-per-head
  contraction is padded to DoubleRow's two K-subtiles with a ZERO weight
  subtile: k8 stores j-blocks interleaved with zero blocks (stride 256), and
  the q rhs second subtile just reads the next 512 columns (zeroed tail).
- softmax exp is split across two engines: even j-tiles on ACT (true Exp,
  applying the 1/sqrt(hd) scale, writing fp8 directly), odd j-tiles on DVE
  via a Schraudolph-style fast exp: bits = round(1.442695*raw + 55.55)
  written as uint8 == the fp8e4m3 encoding of ~exp(raw/8). Softmax averages
  the ~3% per-weight noise over ~2k keys -> ~0.1% output error.
- pv via fp8 DoubleRow with a REAL K=256 contraction (two j-tiles per
  matmul): halves the PE instruction count. The ones column per head
  (row-sum trick) rides along in fp8 (1.0 exact).
- evictions moved off the critical DVE path: qk + norm eviction on ACT
  (Identity-with-bias / Copy read PSUM fine), normalize multiply on GPSIMD
  (SBUF-only), out-projection DMA'd straight from PSUM to DRAM.
"""

import numpy as np

import concourse.bass as bass
import concourse.mybir as mybir
import concourse.tile as tile
from concourse.bass_utils import run_bass_kernel_spmd
from concourse.vector_clock import ScopedClock

B = 2
S = 2048
D = 1024
NH = 16
HD = 64
NCORES = 8
HPC = 4            # heads per core
F32 = mybir.dt.float32
F32R = mybir.dt.float32r
F8 = mybir.dt.float8e4
U8 = mybir.dt.uint8

SCALE = 1.0 / np.sqrt(HD)          # applied inside exp, not folded into Wq
KT = D // 128      # 8 contraction tiles for the projections
NI = S // 512      # 4 i-chunks (query)
NJ = S // 128      # 16 j-tiles (key)
VW = HD + 1        # 65: v columns + ones column
VP = 80            # padded v sub-stride (DoubleRow needs 16B-aligned step)

# "dr4": fp8e4m3 + DoubleRow (2 MACs/cell/cycle, but heavier LDWEIGHTS)
# "e3":  fp8e3m4, no perf mode (bf16-rate ALU, FWL weight loads, +1 mantissa
#        bit -> half the quantization error)
SCORES_MODE = "dr4"
PV_MODE = "e3"
EXP_DVE_JS = frozenset((1, 3, 5, 7, 9, 11, 13, 15))  # j-tiles exp'd on DVE
# Interleave the pair-1 qk projection tiles into the pair-0 attention chunks
# instead of emitting them up front: the ACT queue then never head-of-line
# blocks chunk-0 exps behind evictions whose producer matmuls the scheduler
# deferred as low-priority filler.
INTERLEAVE_PROJ = True
# pv for pair m issues PV_DELAY pairs behind its scores/exp: deeper slack so
# the in-order PE never waits on the exp engines' latency.
PV_DELAY = 2

# Schraudolph fast-exp constants: fp8 bits = round(A*raw_score + B), i.e.
# the log-linear encoding of exp(raw/8). Calibrated numerically (zero mean
# relative error over the score distribution).
EXP_AB = {"dr4": (1.4426950408889634, 55.55), "e3": (2.8853900817779268, 47.08)}


# ---------------------------------------------------------------------------
# Walrus workaround: this toolchain rejects instructions carrying more than
# one sem wait. Split excess waits onto injected same-engine NoOps placed
# directly before the instruction (same-engine program order preserves
# semantics). DMA completion updates are left untouched.
# ---------------------------------------------------------------------------

def _patched_drain_and_barrier(self, tick_clock, wait_clock):
    nc = self.nc
    collector = nc.sync.nop()
    wait_clock.add_sem_waits(
        collector.ins, ScopedClock({None: tick_clock.global_clock})
    )
    si = collector.ins.sync_info
    if si is not None:
        waits = list(si.on_wait or [])
        if len(waits) > 1:
            collector.ins.sync_info = mybir.SyncInfo(
                on_wait=[waits[0]], on_update=list(si.on_update or [])
            )
            for w in waits[1:]:
                n2 = nc.sync.nop()
                n2.ins.sync_info = mybir.SyncInfo(on_wait=[w], on_update=[])
    nc.sync.drain()
    nc.all_engine_barrier()
    popped = nc._tile_sem_poison_stack.pop()
    assert popped is self._sem_poison
    nc.clear_and_free_semaphores(list(self.sems.allocated().values()))
    nc.all_engine_barrier()


def _split_excess_waits(nc, limit=1):
    import bass_rust

    n_split = 0
    for f in nc.m.functions:
        for bb in f.blocks:
            out = []
            for inst in bb.instructions:
                si = inst.sync_info
                waits = list(si.on_wait) if si is not None and si.on_wait else []
                lim = limit
                if len(waits) > lim:
                    for w in waits[:-lim]:
                        nop = bass_rust.InstNoOp(
                            name=f"{inst.name}-waitsplit-{n_split}",
                            ins=[], outs=[], engine=inst.engine,
                        )
                        nop.sync_info = mybir.SyncInfo(on_wait=[w], on_update=[])
                        out.append(nop)
                        n_split += 1
                    inst.sync_info = mybir.SyncInfo(
                        on_wait=waits[-lim:],
                        on_update=list(si.on_update) if si.on_update else [],
                    )
                out.append(inst)
            bb.instructions[:] = out
    return n_split


# ---------------------------------------------------------------------------
# Program build (identical SPMD program on all 8 cores; shards differ in data)
# ---------------------------------------------------------------------------

def _build_program(reps=1, phase="full"):
    """reps>1 repeats the whole computation (idempotently) inside one NEFF —
    used by the benchmark to separate HW exec time from dispatch overhead."""
    tile.TileContext._drain_and_barrier = _patched_drain_and_barrier

    nc = bass.Bass("TRN2", target_bir_lowering=False, debug=False,
                   num_devices=NCORES)

    xT = nc.dram_tensor("xT", [D, S], F32R, kind="ExternalInput").ap()
    wqk = nc.dram_tensor("wqk", [D, 2 * HPC * HD], F32R, kind="ExternalInput").ap()
    bqk = nc.dram_tensor("bqk", [128, 2 * HPC * HD // 128], F32, kind="ExternalInput").ap()
    wv = nc.dram_tensor("wv", [D, HPC * HD], F32R, kind="ExternalInput").ap()
    bvr = nc.dram_tensor("bvr", [128, HPC * HD], F32, kind="ExternalInput").ap()
    ot = nc.dram_tensor("ot", [2 * 128, D], F32R, kind="ExternalInput").ap()
    out_p = nc.dram_tensor("out_p", [S, D], F32, kind="ExternalOutput").ap()
    dbg = None
    if phase != "full":
        # sink for truncated phases: keeps their work live (walrus DCEs
        # side-effect-free loop bodies)
        dbg = nc.dram_tensor("dbg", [128, 8192], F32, kind="ExternalOutput").ap()

    NQK = 2 * HPC * HD // 128     # 4 e-tiles of qk^T

    with tile.TileContext(nc) as tc:
        with (
            nc.allow_low_precision(reason="fp8 attention core"),
            tc.tile_pool(name="weights", bufs=1) as wpool,
            tc.tile_pool(name="acts", bufs=1) as apool,
        ):
            # ---------------- loads ----------------
            xT_sb = [wpool.tile([128, S], F32R, tag=f"xT{k}", name=f"xT{k}")
                     for k in range(KT)]
            wqk_sb = [wpool.tile([128, 2 * HPC * HD], F32R, tag=f"wqk{k}",
                                 name=f"wqk{k}") for k in range(KT)]
            wv_sb = [wpool.tile([128, HPC * HD], F32R, tag=f"wv{k}",
                                name=f"wv{k}") for k in range(KT)]
            bqk_sb = wpool.tile([128, NQK], F32, tag="bqk")
            bvr_sb = wpool.tile([128, HPC * HD], F32, tag="bvr")
            ot_sb = [wpool.tile([128, D], F32R, tag=f"ot{k}", name=f"ot{k}")
                     for k in range(2)]
            nc.sync.dma_start(out=bqk_sb[:], in_=bqk[:])
            nc.sync.dma_start(out=bvr_sb[:], in_=bvr[:])
            for k in range(KT):
                nc.sync.dma_start(out=wqk_sb[k][:], in_=wqk[k * 128:(k + 1) * 128, :])
                nc.sync.dma_start(out=xT_sb[k][:], in_=xT[k * 128:(k + 1) * 128, :])
            for k in range(KT):
                nc.sync.dma_start(out=wv_sb[k][:], in_=wv[k * 128:(k + 1) * 128, :])
            for k in range(2):
                nc.sync.dma_start(out=ot_sb[k][:], in_=ot[k * 128:(k + 1) * 128, :])

            # persistent activation buffers
            # q8[p]: [128, S+512] fp8; partitions: head 2p on 0:64, head 2p+1
            # on 64:128; cols = s; tail cols zeroed (read by the dummy second
            # DoubleRow subtile -> must not contain Inf/NaN encodings).
            # k8[p]: [128, 2*S] fp8; j-block jb at cols 256*jb, zeros at
            # 256*jb+128 (the zero weight subtile for dr4; unused pad for e3).
            # v8[m] (dr4): [128, 4*2*VP]; local head h at 160*h, j-subtile s
            # at +80*s; cols 0:64 values, col 64 = 1.0 (row-sum trick).
            # v8[m] (e3): [128, 4*2*128]; head h at 256*h, j-subtile s at
            # +128*s; cols 0:64 v, col 64 = 1.0, 65:128 zero (pad to a full
            # 128-column weight so FWL triggers).
            F8S = F8 if SCORES_MODE == "dr4" else mybir.dt.float8e3
            F8P = F8 if PV_MODE == "dr4" else mybir.dt.float8e3
            EXP_A, EXP_B = EXP_AB[PV_MODE]
            VSTRIDE = VP if PV_MODE == "dr4" else 128
            q8 = [apool.tile([128, S + 512], F8S, tag=f"q8{p}", name=f"q8{p}")
                  for p in range(2)]
            k8 = [apool.tile([128, 2 * S], F8S, tag=f"k8{p}", name=f"k8{p}")
                  for p in range(2)]
            v8 = [apool.tile([128, HPC * 2 * VSTRIDE], F8P, tag=f"v8{m}",
                             name=f"v8{m}") for m in range(NJ // 2)]
            valsT = [apool.tile([128, S], F32R, tag=f"vals{p}", name=f"vals{p}")
                     for p in range(2)]
            # one-time constant regions (outside the reps loop)
            for p in range(2):
                nc.vector.memset(q8[p][:, S:S + 512], 0.0)
                nc.vector.memset(
                    k8[p][:].rearrange("p (j two n) -> p j two n",
                                       j=NJ, two=2, n=128)[:, :, 1, :], 0.0)
            for m in range(NJ // 2):
                v8v = v8[m][:].rearrange("p (h s c) -> p h s c",
                                         h=HPC, s=2, c=VSTRIDE)
                nc.vector.memset(v8v[:, :, :, HD:HD + 1], 1.0)
                # pad cols: zero (read by the padded e3 weight load; for dr4
                # never read but keep them finite)
                nc.vector.memset(v8v[:, :, :, VW:], 0.0)

            # single psum pool: proj/outproj share tag "pp" (2 banks) +
            # wide (4) + pv (2) = 8 banks, no phase barrier so attention
            # interleaves with late projection tiles.
            with (
                tc.tile_pool(name="ps", bufs=1, space="PSUM") as psp,
                tc.tile_pool(name="attn_sb", bufs=3) as ab,
                tc.tile_pool(name="norm_sb", bufs=2) as nb,
                tc.tile_pool(name="out_sb", bufs=3) as ob,
                tc.tile_pool(name="dram_sc", bufs=4, space="DRAM") as dsc,
            ):
                def emit_qk_tile(t, i, borrow_wide=False):
                    # one qk^T e-tile [e, s-chunk]; bias added on eviction to
                    # fp8 (ACT engine). Early tiles may borrow the (still
                    # idle) "wide" psum slots so more accumulations overlap
                    # the input DMA.
                    if borrow_wide:
                        qs = psp.tile([128, 512], F32, tag="wide", bufs=2,
                                      name="qs")
                    else:
                        qs = psp.tile([128, 512], F32, tag="pp", bufs=2,
                                      name="qs")
                    for k in range(KT):
                        nc.tensor.matmul(
                            qs[:],
                            wqk_sb[k][:, t * 128:(t + 1) * 128],
                            xT_sb[k][:, i * 512:(i + 1) * 512],
                            start=(k == 0), stop=(k == KT - 1),
                        )
                    if t < 2:
                        dst = q8[t][:, i * 512:(i + 1) * 512]
                    else:
                        dst = k8[t - 2][:].rearrange(
                            "p (j two n) -> p j two n", j=NJ, two=2, n=128
                        )[:, 4 * i:4 * i + 4, 0, :]
                    nc.scalar.activation(
                        dst, qs[:], mybir.ActivationFunctionType.Identity,
                        bias=bqk_sb[:, t:t + 1],
                    )

                def emit_qk_proj(ts_pair, borrow_wide=False):
                    n_emitted = 0
                    for i in range(NI):
                        for t in ts_pair:
                            emit_qk_tile(t, i, borrow_wide and n_emitted < 2)
                            n_emitted += 1

                def emit_v_tile(j):
                    # v natural [s, dd] + bias -> fp8 pair layout
                    vs = psp.tile([128, HPC * HD], F32, tag="pp", bufs=2, name="vs")
                    for k in range(KT):
                        nc.tensor.matmul(
                            vs[:],
                            xT_sb[k][:, j * 128:(j + 1) * 128],
                            wv_sb[k][:],
                            start=(k == 0), stop=(k == KT - 1),
                        )
                    v_dst = v8[j // 2][:].rearrange(
                        "p (h s c) -> p h s c", h=HPC, s=2, c=VSTRIDE
                    )[:, :, j % 2, 0:HD]
                    nc.vector.tensor_tensor(
                        v_dst,
                        vs[:].rearrange("p (h e) -> p h e", h=HPC, e=HD),
                        bvr_sb[:].rearrange("p (h e) -> p h e", h=HPC, e=HD),
                        op=mybir.AluOpType.add,
                    )

                def emit_attn(p, i, at_start=(), steps=None):
                    # Software-pipelined over j: scores (fp8 DoubleRow) and
                    # exp (ACT/DVE alternating) per j; pv (fp8 DoubleRow,
                    # K=256) per j-PAIR, one pair behind. `at_start`/`steps`
                    # inject deferred work (normalize / out-proj of the
                    # previous chunk) into this chunk's pipeline.
                    steps = steps or {}
                    isl = slice(i * 512, (i + 1) * 512)
                    for fn in at_start:
                        fn()
                    pvdim = VW if PV_MODE == "dr4" else 128
                    pvA = psp.tile([pvdim, 512], F32, tag="pv", bufs=2, name="pvA")
                    pvB = psp.tile([pvdim, 512], F32, tag="pv", bufs=2, name="pvB")
                    es_q = {}
                    for j in range(NJ + 2 * PV_DELAY):
                        if j < NJ:
                            wide = psp.tile([128, 1024], F32, tag="wide",
                                            bufs=2, name="wide")
                            if SCORES_MODE == "dr4":
                                for h in range(2):
                                    hsl = slice(h * 64, (h + 1) * 64)
                                    lhsT = k8[p][hsl, :].rearrange(
                                        "p (j two n) -> p j two n",
                                        j=NJ, two=2, n=128)[:, j]
                                    rhs = q8[p][hsl, i * 512:i * 512 + 1024] \
                                        .rearrange("p (two n) -> p two n",
                                                   two=2, n=512)
                                    nc.tensor.matmul(
                                        wide[:, h * 512:(h + 1) * 512],
                                        lhsT, rhs, start=True, stop=True,
                                        perf_mode=mybir.MatmulPerfMode.DoubleRow,
                                    )
                            else:
                                for h in range(2):
                                    hsl = slice(h * 64, (h + 1) * 64)
                                    nc.tensor.matmul(
                                        wide[:, h * 512:(h + 1) * 512],
                                        k8[p][hsl, :].rearrange(
                                            "p (j two n) -> p j two n",
                                            j=NJ, two=2, n=128)[:, j, 0, :],
                                        q8[p][hsl, isl],
                                        start=True, stop=True)
                            # exp -> fp8 es, pair layout:
                            # [hA s0 | hA s1 | hB s0 | hB s1] x 512
                            if j % 2 == 0:
                                es = ab.tile([128, 2048], F8P, tag="es",
                                             bufs=4, name="es")
                                es_q[j // 2] = es
                            else:
                                es = es_q[j // 2]
                            es_dst = es[:].rearrange(
                                "p (h s n) -> p h s n", h=2, s=2, n=512
                            )[:, :, j % 2, :]
                            if j in EXP_DVE_JS:
                                nc.vector.tensor_scalar(
                                    es_dst.bitcast(U8), wide[:],
                                    EXP_A, EXP_B,
                                    op0=mybir.AluOpType.mult,
                                    op1=mybir.AluOpType.add,
                                )
                            else:
                                nc.scalar.activation(
                                    es_dst, wide[:],
                                    mybir.ActivationFunctionType.Exp,
                                    scale=SCALE,
                                )
                        if j % 2 == 0 and j >= 2 * PV_DELAY:
                            m = j // 2 - PV_DELAY
                            es = es_q.pop(m)
                            for h, pv in ((0, pvA), (1, pvB)):
                                v8v = v8[m][:].rearrange(
                                    "p (h s c) -> p h s c", h=HPC, s=2,
                                    c=VSTRIDE)
                                rhs = es[:, h * 1024:(h + 1) * 1024] \
                                    .rearrange("p (two n) -> p two n",
                                               two=2, n=512)
                                if PV_MODE == "dr4":
                                    nc.tensor.matmul(
                                        pv[:], v8v[:, 2 * p + h, :, 0:VW], rhs,
                                        start=(m == 0), stop=(m == NJ // 2 - 1),
                                        perf_mode=mybir.MatmulPerfMode.DoubleRow,
                                    )
                                else:
                                    for s in range(2):
                                        nc.tensor.matmul(
                                            pv[:], v8v[:, 2 * p + h, s, :],
                                            rhs[:, s, :],
                                            start=(m == 0 and s == 0),
                                            stop=(m == NJ // 2 - 1 and s == 1),
                                        )
                        for fn in steps.get(j, ()):
                            fn()
                    return pvA, pvB

                def norm_evict(pvA, pvB):
                    # evict pv psum banks asap (ACT) so the next chunk can
                    # claim them
                    raws = []
                    for pv in (pvA, pvB):
                        raw = nb.tile([VW, 512], F32, tag="raw", bufs=3,
                                      name="raw")
                        nc.scalar.copy(raw[:], pv[0:VW, :])
                        raws.append(raw)
                    return raws

                def norm_recip(raws):
                    # reciprocal of the row-sum rows (DVE, one lane)
                    recips = []
                    for raw in raws:
                        recip = nb.tile([1, 512], F32R, tag="recip", bufs=2,
                                        name="recip")
                        nc.vector.reciprocal(recip[:], raw[HD:VW, :])
                        recips.append(recip)
                    return recips

                def norm_bcmm(recips):
                    # broadcast recip across 64 partitions via a DRAM
                    # round-trip (stride-0 read DMA); on the ACT hwdge ring
                    # so the bulk out_p writes on the SP ring can't delay it
                    bcs = []
                    for recip in recips:
                        rd = dsc.tile([1, 512], F32R, tag="rd", name="rd")
                        nc.scalar.dma_start(out=rd[:], in_=recip[:])
                        bc = nb.tile([64, 512], F32R, tag="bc", bufs=3,
                                     name="bc")
                        nc.scalar.dma_start(out=bc[:],
                                            in_=rd[:].to_broadcast((64, 512)))
                        bcs.append(bc)
                    return bcs

                def norm_stt(p, i, raws, bcs):
                    # normalize on GPSIMD (SBUF-only engine, otherwise idle)
                    isl = slice(i * 512, (i + 1) * 512)
                    for hh, (raw, bc) in enumerate(zip(raws, bcs)):
                        nc.gpsimd.tensor_tensor(
                            valsT[p][hh * 64:(hh + 1) * 64, isl],
                            raw[0:HD, :], bc[:],
                            op=mybir.AluOpType.mult,
                        )

                def norm_rest(p, i, raws):
                    norm_stt(p, i, raws, norm_bcmm(norm_recip(raws)))

                def emit_outproj(i, si_range=range(4)):
                    # out projection for this i-chunk; eviction alternates
                    # ACT/DVE to balance engine load
                    for si in si_range:
                        s0 = i * 512 + si * 128
                        for e in range(2):
                            op = psp.tile([128, 512], F32, tag="pp", bufs=2, name="op")
                            for k in range(2):
                                nc.tensor.matmul(
                                    op[:],
                                    valsT[k][:, s0:s0 + 128],
                                    ot_sb[k][:, e * 512:(e + 1) * 512],
                                    start=(k == 0), stop=(k == 1),
                                )
                            osb = ob.tile([128, 512], F32, tag="osb", bufs=6, name="osb")
                            if (si + e) % 2 == 0:
                                nc.scalar.copy(osb[:], op[:])
                            else:
                                nc.vector.tensor_copy(osb[:], op[:])
                            nc.sync.dma_start(
                                out=out_p[s0:s0 + 128, e * 512:(e + 1) * 512],
                                in_=osb[:],
                            )

                # Projections are emitted first (dep-tracking needs
                # program order = data order), but attention + out-proj get a
                # low priority band so the scheduler treats projection work
                # as filler once attention tiles become data-ready.
                attn_base = [1]

                def prio():
                    return tc.high_priority(offset=tc.cur_priority - attn_base[0])

                def emit_body():
                    # per i-chunk: the two qk e-tiles pair-0 scores need,
                    # then the 4 v j-tiles that chunk's pv consumes — so
                    # attention chunk (0, 0) unblocks after ~6 proj tiles
                    # instead of after the whole projection phase.
                    for i in range(NI):
                        emit_qk_tile(0, i, borrow_wide=(i == 0))
                        emit_qk_tile(2, i, borrow_wide=(i == 0))
                        for j in range(4 * i, 4 * i + 4):
                            emit_v_tile(j)
                    if not INTERLEAVE_PROJ or phase == "proj":
                        emit_qk_proj((1, 3))
                    if phase == "proj":
                        return
                    chunks = [(p, i) for p in range(2) for i in range(NI)]
                    pend = None        # previous chunk awaiting normalize
                    for (p, i) in chunks:
                        at_start = []
                        steps = {}
                        if INTERLEAVE_PROJ and p == 0:
                            steps.setdefault(5, []).append(
                                lambda i2=i: emit_qk_tile(1, i2))
                            steps.setdefault(13, []).append(
                                lambda i2=i: emit_qk_tile(3, i2))
                        if pend is not None:
                            pp_, ii_, pvA_, pvB_ = pend
                            box = []
                            bcbox = []
                            at_start.append(
                                lambda a=pvA_, b=pvB_, bx=box: bx.extend(
                                    norm_evict(a, b)))
                            rbox = []
                            steps.setdefault(1, []).append(
                                lambda bx=box, rb=rbox: rb.extend(
                                    norm_recip(bx)))
                            steps.setdefault(2, []).append(
                                lambda rb=rbox, bb=bcbox: bb.extend(
                                    norm_bcmm(rb)))
                            steps.setdefault(6, []).append(
                                lambda p2=pp_, i2=ii_, bx=box, bb=bcbox:
                                    norm_stt(p2, i2, bx, bb))
                            if pp_ == 1 and phase == "full":
                                # out-projection for ii_ right after its stt
                                for k, si in enumerate((8, 10, 12, 14)):
                                    steps.setdefault(si, []).append(
                                        lambda i2=ii_, k2=k: emit_outproj(
                                            i2, range(k2, k2 + 1)))
                        with prio():
                            pvA, pvB = emit_attn(p, i, at_start, steps)
                            attn_base[0] = tc.cur_priority
                        pend = (p, i, pvA, pvB)
                    # tail: last chunk's normalize + final out-projections
                    with prio():
                        pp_, ii_, pvA_, pvB_ = pend
                        raws = norm_evict(pvA_, pvB_)
                        norm_rest(pp_, ii_, raws)
                        if phase == "full":
                            emit_outproj(ii_)
                        attn_base[0] = tc.cur_priority

                if reps == 1:
                    emit_body()
                else:
                    with tc.For_i(0, reps, 1):
                        emit_body()
                if phase == "proj":
                    nc.sync.dma_start(out=dbg[:, 0:640],
                                      in_=q8[0][:].bitcast(F32))
                    nc.sync.dma_start(out=dbg[:, 640:1280],
                                      in_=q8[1][:].bitcast(F32))
                    nc.sync.dma_start(out=dbg[:, 1280:2304],
                                      in_=k8[0][:].bitcast(F32))
                    nc.sync.dma_start(out=dbg[:, 2304:3328],
                                      in_=k8[1][:].bitcast(F32))
                    for m in range(NJ // 2):
                        w = HPC * 2 * VSTRIDE // 4
                        nc.sync.dma_start(
                            out=dbg[:, 3328 + m * w:3328 + (m + 1) * w],
                            in_=v8[m][:].bitcast(F32))
                elif phase == "noout":
                    for k in range(2):
                        nc.sync.dma_start(
                            out=dbg[:, k * 2048:(k + 1) * 2048],
                            in_=valsT[k][:].bitcast(F32))

    _split_excess_waits(nc)
    return nc


_program_cache = None


def _get_program():
    global _program_cache
    if _program_cache is None:
        _program_cache = _build_program()
    return _program_cache


# ---------------------------------------------------------------------------
# Host-side sharding + gather
# ---------------------------------------------------------------------------

def _shard_inputs(x, qkv_w, qkv_b, out_w):
    """Build the 8 per-core input maps."""
    x = np.asarray(x, np.float32)
    qkv_w = np.asarray(qkv_w, np.float32)
    qkv_b = np.asarray(qkv_b, np.float32)
    out_w = np.asarray(out_w, np.float32)

    # per-head q/k/v rows of the fused projection: head h covers rows
    # [h*3*HD, (h+1)*3*HD) split q | k | v
    qw = np.stack([qkv_w[h * 3 * HD: h * 3 * HD + HD] for h in range(NH)])
    kw = np.stack([qkv_w[h * 3 * HD + HD: h * 3 * HD + 2 * HD] for h in range(NH)])
    vw = np.stack([qkv_w[h * 3 * HD + 2 * HD: h * 3 * HD + 3 * HD] for h in range(NH)])
    qb = np.stack([qkv_b[h * 3 * HD: h * 3 * HD + HD] for h in range(NH)])
    kb = np.stack([qkv_b[h * 3 * HD + HD: h * 3 * HD + 2 * HD] for h in range(NH)])
    vb = np.stack([qkv_b[h * 3 * HD + 2 * HD: h * 3 * HD + 3 * HD] for h in range(NH)])

    xT = [np.ascontiguousarray(x[b].T) for b in range(B)]

    in_maps = []
    for c in range(NCORES):
        b = c // HPC
        g = c % HPC
        hs = slice(g * HPC, (g + 1) * HPC)
        # [4, HD, D] -> [4*HD, D]; NO softmax scale folding (exp applies it)
        Wq = qw[hs].reshape(HPC * HD, D)
        Wk = kw[hs].reshape(HPC * HD, D)
        Wv = vw[hs].reshape(HPC * HD, D)
        bq = qb[hs].reshape(HPC * HD)
        bk = kb[hs].reshape(HPC * HD)
        bv = vb[hs].reshape(HPC * HD)

        wqk_c = np.ascontiguousarray(np.concatenate([Wq, Wk], 0).T)  # [D, 512]
        bqk_full = np.concatenate([bq, bk])                          # [512]
        bqk_c = np.ascontiguousarray(bqk_full.reshape(-1, 128).T)    # [128, 4]
        wv_c = np.ascontiguousarray(Wv.T)                            # [D, 256]
        bvr_c = np.ascontiguousarray(np.broadcast_to(bv, (128, HPC * HD)))
        # out_w columns for these heads, transposed: [256, D]
        cols = np.arange(g * HPC * HD, (g + 1) * HPC * HD)
        ot_c = np.ascontiguousarray(out_w[:, cols].T)

        in_maps.append({
            "xT": xT[b],
            "wqk": wqk_c,
            "bqk": bqk_c,
            "wv": wv_c,
            "bvr": bvr_c,
            "ot": ot_c,
        })
    return in_maps


def kernel(x, qkv_w, qkv_b, out_w, out_b):
    nc = _get_program()
    in_maps = _shard_inputs(x, qkv_w, qkv_b, out_w)
    res = run_bass_kernel_spmd(nc, in_maps, core_ids=list(range(NCORES)))
    parts = [res.results[c]["out_p"] for c in range(NCORES)]
    out_b = np.asarray(out_b, np.float32)
    out = np.empty((B, S, D), np.float32)
    for b in range(B):
        acc = np.zeros((S, D), np.float64)
        for g in range(HPC):
            acc += parts[b * HPC + g]
        out[b] = (acc + out_b[None, :]).astype(np.float32)
    return out


# revision 25
# speedup vs baseline: 1.0433x; 1.0433x over previous
"""Multi-head attention (B=2, S=2048, D=1024, H=16, hd=64) on 8 TRN2 cores.

Sharding: tensor-parallel on heads x data-parallel on batch. Core c handles
batch b=c//4 and heads [4*(c%4), 4*(c%4)+4). Each core computes the fused QKV
projection for its head slice, attention for its 4 heads, and a partial
out-projection (out_w column slice); the host sums the 4 partials per batch
and adds out_b.

fp8 attention core (v2):
- q/k/v are quantized to fp8e4m3 on eviction from the projection psum.
- scores^T via fp8 DoubleRow matmuls (2 MACs/cell/cycle). The K=64-per-head
  contraction is padded to DoubleRow's two K-subtiles with a ZERO weight
  subtile: k8 stores j-blocks interleaved with zero blocks (stride 256), and
  the q rhs second subtile just reads the next 512 columns (zeroed tail).
- softmax exp is split across two engines: even j-tiles on ACT (true Exp,
  applying the 1/sqrt(hd) scale, writing fp8 directly), odd j-tiles on DVE
  via a Schraudolph-style fast exp: bits = round(1.442695*raw + 55.55)
  written as uint8 == the fp8e4m3 encoding of ~exp(raw/8). Softmax averages
  the ~3% per-weight noise over ~2k keys -> ~0.1% output error.
- pv via fp8 DoubleRow with a REAL K=256 contraction (two j-tiles per
  matmul): halves the PE instruction count. The ones column per head
  (row-sum trick) rides along in fp8 (1.0 exact).
- evictions moved off the critical DVE path: qk + norm eviction on ACT
  (Identity-with-bias / Copy read PSUM fine), normalize multiply on GPSIMD
  (SBUF-only), out-projection DMA'd straight from PSUM to DRAM.
"""

import numpy as np

import concourse.bass as bass
import concourse.mybir as mybir
import concourse.tile as tile
from concourse.bass_utils import run_bass_kernel_spmd
from concourse.vector_clock import ScopedClock

B = 2
S = 2048
D = 1024
NH = 16
HD = 64
NCORES = 8
HPC = 4            # heads per core
F32 = mybir.dt.float32
F32R = mybir.dt.float32r
F8 = mybir.dt.float8e4
BF16 = mybir.dt.bfloat16
U8 = mybir.dt.uint8

SCALE = 1.0 / np.sqrt(HD)          # applied inside exp, not folded into Wq
KT = D // 128      # 8 contraction tiles for the projections
NI = S // 512      # 4 i-chunks (query)
NJ = S // 128      # 16 j-tiles (key)
VW = HD + 1        # 65: v columns + ones column
VP = 80            # padded v sub-stride (DoubleRow needs 16B-aligned step)

# "dr4": fp8e4m3 + DoubleRow (2 MACs/cell/cycle, but heavier LDWEIGHTS)
# "e3":  fp8e3m4, no perf mode (bf16-rate ALU, FWL weight loads, +1 mantissa
#        bit -> half the quantization error)
SCORES_MODE = "e3"
PV_MODE = "e3"
EXP_DVE_JS = frozenset((1, 3, 5, 7, 9, 11, 13, 15))  # j-tiles exp'd on DVE
# Interleave the pair-1 qk projection tiles into the pair-0 attention chunks
# instead of emitting them up front: the ACT queue then never head-of-line
# blocks chunk-0 exps behind evictions whose producer matmuls the scheduler
# deferred as low-priority filler.
INTERLEAVE_PROJ = True
# pv for pair m issues PV_DELAY pairs behind its scores/exp: deeper slack so
# the in-order PE never waits on the exp engines' latency.
PV_DELAY = 2

# Schraudolph fast-exp constants: fp8 bits = round(A*raw_score + B), i.e.
# the log-linear encoding of exp(raw/8). Calibrated numerically (zero mean
# relative error over the score distribution).
EXP_AB = {"dr4": (1.4426950408889634, 55.55), "e3": (2.8853900817779268, 47.08)}


# ---------------------------------------------------------------------------
# Walrus workaround: this toolchain rejects instructions carrying more than
# one sem wait. Split excess waits onto injected same-engine NoOps placed
# directly before the instruction (same-engine program order preserves
# semantics). DMA completion updates are left untouched.
# ---------------------------------------------------------------------------

def _patched_drain_and_barrier(self, tick_clock, wait_clock):
    nc = self.nc
    collector = nc.sync.nop()
    wait_clock.add_sem_waits(
        collector.ins, ScopedClock({None: tick_clock.global_clock})
    )
    si = collector.ins.sync_info
    if si is not None:
        waits = list(si.on_wait or [])
        if len(waits) > 1:
            collector.ins.sync_info = mybir.SyncInfo(
                on_wait=[waits[0]], on_update=list(si.on_update or [])
            )
            for w in waits[1:]:
                n2 = nc.sync.nop()
                n2.ins.sync_info = mybir.SyncInfo(on_wait=[w], on_update=[])
    nc.sync.drain()
    nc.all_engine_barrier()
    popped = nc._tile_sem_poison_stack.pop()
    assert popped is self._sem_poison
    nc.clear_and_free_semaphores(list(self.sems.allocated().values()))
    nc.all_engine_barrier()


def _split_excess_waits(nc, limit=1):
    import bass_rust

    n_split = 0
    for f in nc.m.functions:
        for bb in f.blocks:
            out = []
            for inst in bb.instructions:
                si = inst.sync_info
                waits = list(si.on_wait) if si is not None and si.on_wait else []
                lim = limit
                if len(waits) > lim:
                    for w in waits[:-lim]:
                        nop = bass_rust.InstNoOp(
                            name=f"{inst.name}-waitsplit-{n_split}",
                            ins=[], outs=[], engine=inst.engine,
                        )
                        nop.sync_info = mybir.SyncInfo(on_wait=[w], on_update=[])
                        out.append(nop)
                        n_split += 1
                    inst.sync_info = mybir.SyncInfo(
                        on_wait=waits[-lim:],
                        on_update=list(si.on_update) if si.on_update else [],
                    )
                out.append(inst)
            bb.instructions[:] = out
    return n_split


# ---------------------------------------------------------------------------
# Program build (identical SPMD program on all 8 cores; shards differ in data)
# ---------------------------------------------------------------------------

def _build_program(reps=1, phase="full"):
    """reps>1 repeats the whole computation (idempotently) inside one NEFF —
    used by the benchmark to separate HW exec time from dispatch overhead."""
    tile.TileContext._drain_and_barrier = _patched_drain_and_barrier

    nc = bass.Bass("TRN2", target_bir_lowering=False, debug=False,
                   num_devices=NCORES)

    xT = nc.dram_tensor("xT", [D, S], F32R, kind="ExternalInput").ap()
    wqk = nc.dram_tensor("wqk", [D, 2 * HPC * HD], F32R, kind="ExternalInput").ap()
    bqk = nc.dram_tensor("bqk", [128, 2 * HPC * HD // 128], F32, kind="ExternalInput").ap()
    wv = nc.dram_tensor("wv", [D, HPC * HD], F32R, kind="ExternalInput").ap()
    bvr = nc.dram_tensor("bvr", [128, HPC * HD], F32, kind="ExternalInput").ap()
    ot = nc.dram_tensor("ot", [2 * 128, D], F32R, kind="ExternalInput").ap()
    out_p = nc.dram_tensor("out_p", [S, D], BF16, kind="ExternalOutput").ap()
    dbg = None
    if phase != "full":
        # sink for truncated phases: keeps their work live (walrus DCEs
        # side-effect-free loop bodies)
        dbg = nc.dram_tensor("dbg", [128, 8192], F32, kind="ExternalOutput").ap()

    NQK = 2 * HPC * HD // 128     # 4 e-tiles of qk^T

    with tile.TileContext(nc) as tc:
        with (
            nc.allow_low_precision(reason="fp8 attention core"),
            tc.tile_pool(name="weights", bufs=1) as wpool,
            tc.tile_pool(name="acts", bufs=1) as apool,
        ):
            # ---------------- loads ----------------
            xT_sb = [wpool.tile([128, S], F32R, tag=f"xT{k}", name=f"xT{k}")
                     for k in range(KT)]
            wqk_sb = [wpool.tile([128, 2 * HPC * HD], F32R, tag=f"wqk{k}",
                                 name=f"wqk{k}") for k in range(KT)]
            wv_sb = [wpool.tile([128, HPC * HD], F32R, tag=f"wv{k}",
                                name=f"wv{k}") for k in range(KT)]
            bqk_sb = wpool.tile([128, NQK], F32, tag="bqk")
            bvr_sb = wpool.tile([128, HPC * HD], F32, tag="bvr")
            ot_sb = [wpool.tile([128, D], F32R, tag=f"ot{k}", name=f"ot{k}")
                     for k in range(2)]
            nc.sync.dma_start(out=bqk_sb[:], in_=bqk[:])
            nc.sync.dma_start(out=bvr_sb[:], in_=bvr[:])
            for k in range(KT):
                nc.sync.dma_start(out=wqk_sb[k][:], in_=wqk[k * 128:(k + 1) * 128, :])
                nc.sync.dma_start(out=xT_sb[k][:], in_=xT[k * 128:(k + 1) * 128, :])
            for k in range(KT):
                nc.sync.dma_start(out=wv_sb[k][:], in_=wv[k * 128:(k + 1) * 128, :])
            for k in range(2):
                nc.sync.dma_start(out=ot_sb[k][:], in_=ot[k * 128:(k + 1) * 128, :])

            # persistent activation buffers
            # q8[p]: [128, S+512] fp8; partitions: head 2p on 0:64, head 2p+1
            # on 64:128; cols = s; tail cols zeroed (read by the dummy second
            # DoubleRow subtile -> must not contain Inf/NaN encodings).
            # k8[p]: [128, 2*S] fp8; j-block jb at cols 256*jb, zeros at
            # 256*jb+128 (the zero weight subtile for dr4; unused pad for e3).
            # v8[m] (dr4): [128, 4*2*VP]; local head h at 160*h, j-subtile s
            # at +80*s; cols 0:64 values, col 64 = 1.0 (row-sum trick).
            # v8[m] (e3): [128, 4*2*128]; head h at 256*h, j-subtile s at
            # +128*s; cols 0:64 v, col 64 = 1.0, 65:128 zero (pad to a full
            # 128-column weight so FWL triggers).
            F8S = F8 if SCORES_MODE == "dr4" else mybir.dt.float8e3
            F8P = F8 if PV_MODE == "dr4" else mybir.dt.float8e3
            EXP_A, EXP_B = EXP_AB[PV_MODE]
            VSTRIDE = VP if PV_MODE == "dr4" else 128
            q8 = [apool.tile([128, S + 512], F8S, tag=f"q8{p}", name=f"q8{p}")
                  for p in range(2)]
            k8 = [apool.tile([128, 2 * S], F8S, tag=f"k8{p}", name=f"k8{p}")
                  for p in range(2)]
            v8 = [apool.tile([128, HPC * 2 * VSTRIDE], F8P, tag=f"v8{m}",
                             name=f"v8{m}") for m in range(NJ // 2)]
            valsT = [apool.tile([128, S], F32R, tag=f"vals{p}", name=f"vals{p}")
                     for p in range(2)]
            # one-time constant regions (outside the reps loop)
            for p in range(2):
                nc.vector.memset(q8[p][:, S:S + 512], 0.0)
                nc.vector.memset(
                    k8[p][:].rearrange("p (j two n) -> p j two n",
                                       j=NJ, two=2, n=128)[:, :, 1, :], 0.0)
            for m in range(NJ // 2):
                v8v = v8[m][:].rearrange("p (h s c) -> p h s c",
                                         h=HPC, s=2, c=VSTRIDE)
                nc.vector.memset(v8v[:, :, :, HD:HD + 1], 1.0)
                # pad cols: zero (read by the padded e3 weight load; for dr4
                # never read but keep them finite)
                nc.vector.memset(v8v[:, :, :, VW:], 0.0)

            # single psum pool: proj/outproj share tag "pp" (2 banks) +
            # wide (4) + pv (2) = 8 banks, no phase barrier so attention
            # interleaves with late projection tiles.
            with (
                tc.tile_pool(name="ps", bufs=1, space="PSUM") as psp,
                tc.tile_pool(name="attn_sb", bufs=3) as ab,
                tc.tile_pool(name="norm_sb", bufs=2) as nb,
                tc.tile_pool(name="out_sb", bufs=3) as ob,
                tc.tile_pool(name="dram_sc", bufs=4, space="DRAM") as dsc,
            ):
                def emit_qk_tile(t, i, borrow_wide=False):
                    # one qk^T e-tile [e, s-chunk]; bias added on eviction to
                    # fp8 (ACT engine). Early tiles may borrow the (still
                    # idle) "wide" psum slots so more accumulations overlap
                    # the input DMA.
                    if borrow_wide:
                        qs = psp.tile([128, 512], F32, tag="wide", bufs=2,
                                      name="qs")
                    else:
                        qs = psp.tile([128, 512], F32, tag="pp", bufs=2,
                                      name="qs")
                    for k in range(KT):
                        nc.tensor.matmul(
                            qs[:],
                            wqk_sb[k][:, t * 128:(t + 1) * 128],
                            xT_sb[k][:, i * 512:(i + 1) * 512],
                            start=(k == 0), stop=(k == KT - 1),
                        )
                    if t < 2:
                        dst = q8[t][:, i * 512:(i + 1) * 512]
                    else:
                        dst = k8[t - 2][:].rearrange(
                            "p (j two n) -> p j two n", j=NJ, two=2, n=128
                        )[:, 4 * i:4 * i + 4, 0, :]
                    nc.scalar.activation(
                        dst, qs[:], mybir.ActivationFunctionType.Identity,
                        bias=bqk_sb[:, t:t + 1],
                    )

                def emit_qk_proj(ts_pair, borrow_wide=False):
                    n_emitted = 0
                    for i in range(NI):
                        for t in ts_pair:
                            emit_qk_tile(t, i, borrow_wide and n_emitted < 2)
                            n_emitted += 1

                def emit_v_tile(j):
                    # v natural [s, dd] + bias -> fp8 pair layout
                    vs = psp.tile([128, HPC * HD], F32, tag="pp", bufs=2, name="vs")
                    for k in range(KT):
                        nc.tensor.matmul(
                            vs[:],
                            xT_sb[k][:, j * 128:(j + 1) * 128],
                            wv_sb[k][:],
                            start=(k == 0), stop=(k == KT - 1),
                        )
                    v_dst = v8[j // 2][:].rearrange(
                        "p (h s c) -> p h s c", h=HPC, s=2, c=VSTRIDE
                    )[:, :, j % 2, 0:HD]
                    nc.vector.tensor_tensor(
                        v_dst,
                        vs[:].rearrange("p (h e) -> p h e", h=HPC, e=HD),
                        bvr_sb[:].rearrange("p (h e) -> p h e", h=HPC, e=HD),
                        op=mybir.AluOpType.add,
                    )

                def emit_attn(p, i, at_start=(), steps=None):
                    # Software-pipelined over j: scores (fp8 DoubleRow) and
                    # exp (ACT/DVE alternating) per j; pv (fp8 DoubleRow,
                    # K=256) per j-PAIR, one pair behind. `at_start`/`steps`
                    # inject deferred work (normalize / out-proj of the
                    # previous chunk) into this chunk's pipeline.
                    steps = steps or {}
                    isl = slice(i * 512, (i + 1) * 512)
                    for fn in at_start:
                        fn()
                    pvdim = VW if PV_MODE == "dr4" else 128
                    pvA = psp.tile([pvdim, 512], F32, tag="pv", bufs=2, name="pvA")
                    pvB = psp.tile([pvdim, 512], F32, tag="pv", bufs=2, name="pvB")
                    es_q = {}
                    for j in range(NJ + 2 * PV_DELAY):
                        if j < NJ:
                            wide = psp.tile([128, 1024], F32, tag="wide",
                                            bufs=2, name="wide")
                            if SCORES_MODE == "dr4":
                                for h in range(2):
                                    hsl = slice(h * 64, (h + 1) * 64)
                                    lhsT = k8[p][hsl, :].rearrange(
                                        "p (j two n) -> p j two n",
                                        j=NJ, two=2, n=128)[:, j]
                                    rhs = q8[p][hsl, i * 512:i * 512 + 1024] \
                                        .rearrange("p (two n) -> p two n",
                                                   two=2, n=512)
                                    nc.tensor.matmul(
                                        wide[:, h * 512:(h + 1) * 512],
                                        lhsT, rhs, start=True, stop=True,
                                        perf_mode=mybir.MatmulPerfMode.DoubleRow,
                                    )
                            else:
                                for h in range(2):
                                    hsl = slice(h * 64, (h + 1) * 64)
                                    nc.tensor.matmul(
                                        wide[:, h * 512:(h + 1) * 512],
                                        k8[p][hsl, :].rearrange(
                                            "p (j two n) -> p j two n",
                                            j=NJ, two=2, n=128)[:, j, 0, :],
                                        q8[p][hsl, isl],
                                        start=True, stop=True)
                            # exp -> fp8 es, pair layout:
                            # [hA s0 | hA s1 | hB s0 | hB s1] x 512
                            if j % 2 == 0:
                                es = ab.tile([128, 2048], F8P, tag="es",
                                             bufs=4, name="es")
                                es_q[j // 2] = es
                            else:
                                es = es_q[j // 2]
                            es_dst = es[:].rearrange(
                                "p (h s n) -> p h s n", h=2, s=2, n=512
                            )[:, :, j % 2, :]
                            if j in EXP_DVE_JS:
                                nc.vector.tensor_scalar(
                                    es_dst.bitcast(U8), wide[:],
                                    EXP_A, EXP_B,
                                    op0=mybir.AluOpType.mult,
                                    op1=mybir.AluOpType.add,
                                )
                            else:
                                nc.scalar.activation(
                                    es_dst, wide[:],
                                    mybir.ActivationFunctionType.Exp,
                                    scale=SCALE,
                                )
                        if j % 2 == 0 and j >= 2 * PV_DELAY:
                            m = j // 2 - PV_DELAY
                            es = es_q.pop(m)
                            for h, pv in ((0, pvA), (1, pvB)):
                                v8v = v8[m][:].rearrange(
                                    "p (h s c) -> p h s c", h=HPC, s=2,
                                    c=VSTRIDE)
                                rhs = es[:, h * 1024:(h + 1) * 1024] \
                                    .rearrange("p (two n) -> p two n",
                                               two=2, n=512)
                                if PV_MODE == "dr4":
                                    nc.tensor.matmul(
                                        pv[:], v8v[:, 2 * p + h, :, 0:VW], rhs,
                                        start=(m == 0), stop=(m == NJ // 2 - 1),
                                        perf_mode=mybir.MatmulPerfMode.DoubleRow,
                                    )
                                else:
                                    for s in range(2):
                                        nc.tensor.matmul(
                                            pv[:], v8v[:, 2 * p + h, s, :],
                                            rhs[:, s, :],
                                            start=(m == 0 and s == 0),
                                            stop=(m == NJ // 2 - 1 and s == 1),
                                        )
                        for fn in steps.get(j, ()):
                            fn()
                    return pvA, pvB

                def norm_evict(pvA, pvB):
                    # evict pv psum banks asap (ACT) so the next chunk can
                    # claim them
                    raws = []
                    for pv in (pvA, pvB):
                        raw = nb.tile([VW, 512], F32, tag="raw", bufs=3,
                                      name="raw")
                        nc.scalar.copy(raw[:], pv[0:VW, :])
                        raws.append(raw)
                    return raws

                def norm_recip(raws):
                    # reciprocal of the row-sum rows (DVE, one lane)
                    recips = []
                    for raw in raws:
                        recip = nb.tile([1, 512], F32R, tag="recip", bufs=2,
                                        name="recip")
                        nc.vector.reciprocal(recip[:], raw[HD:VW, :])
                        recips.append(recip)
                    return recips

                def norm_bcmm(recips):
                    # broadcast recip across 64 partitions via a DRAM
                    # round-trip (stride-0 read DMA); on the ACT hwdge ring
                    # so the bulk out_p writes on the SP ring can't delay it
                    bcs = []
                    for recip in recips:
                        rd = dsc.tile([1, 512], F32R, tag="rd", name="rd")
                        nc.scalar.dma_start(out=rd[:], in_=recip[:])
                        bc = nb.tile([64, 512], F32R, tag="bc", bufs=3,
                                     name="bc")
                        nc.scalar.dma_start(out=bc[:],
                                            in_=rd[:].to_broadcast((64, 512)))
                        bcs.append(bc)
                    return bcs

                def norm_stt(p, i, raws, bcs):
                    # normalize on GPSIMD (SBUF-only engine, otherwise idle)
                    isl = slice(i * 512, (i + 1) * 512)
                    for hh, (raw, bc) in enumerate(zip(raws, bcs)):
                        nc.gpsimd.tensor_tensor(
                            valsT[p][hh * 64:(hh + 1) * 64, isl],
                            raw[0:HD, :], bc[:],
                            op=mybir.AluOpType.mult,
                        )

                def norm_rest(p, i, raws):
                    norm_stt(p, i, raws, norm_bcmm(norm_recip(raws)))

                def emit_outproj(i, si_range=range(4)):
                    # out projection for this i-chunk; eviction alternates
                    # ACT/DVE to balance engine load
                    for si in si_range:
                        s0 = i * 512 + si * 128
                        for e in range(2):
                            op = psp.tile([128, 512], F32, tag="pp", bufs=2, name="op")
                            for k in range(2):
                                nc.tensor.matmul(
                                    op[:],
                                    valsT[k][:, s0:s0 + 128],
                                    ot_sb[k][:, e * 512:(e + 1) * 512],
                                    start=(k == 0), stop=(k == 1),
                                )
                            osb = ob.tile([128, 512], BF16, tag="osb", bufs=6, name="osb")
                            if (si + e) % 2 == 0:
                                nc.scalar.copy(osb[:], op[:])
                            else:
                                nc.vector.tensor_copy(osb[:], op[:])
                            nc.sync.dma_start(
                                out=out_p[s0:s0 + 128, e * 512:(e + 1) * 512],
                                in_=osb[:],
                            )

                # Projections are emitted first (dep-tracking needs
                # program order = data order), but attention + out-proj get a
                # low priority band so the scheduler treats projection work
                # as filler once attention tiles become data-ready.
                attn_base = [1]

                def prio():
                    return tc.high_priority(offset=tc.cur_priority - attn_base[0])

                def emit_body():
                    # per i-chunk: the two qk e-tiles pair-0 scores need,
                    # then the 4 v j-tiles that chunk's pv consumes — so
                    # attention chunk (0, 0) unblocks after ~6 proj tiles
                    # instead of after the whole projection phase.
                    for i in range(NI):
                        emit_qk_tile(0, i, borrow_wide=(i == 0))
                        emit_qk_tile(2, i, borrow_wide=(i == 0))
                        for j in range(4 * i, 4 * i + 4):
                            emit_v_tile(j)
                    if not INTERLEAVE_PROJ or phase == "proj":
                        emit_qk_proj((1, 3))
                    if phase == "proj":
                        return
                    chunks = [(p, i) for p in range(2) for i in range(NI)]
                    pend = None        # previous chunk awaiting normalize
                    for (p, i) in chunks:
                        at_start = []
                        steps = {}
                        if INTERLEAVE_PROJ and p == 0:
                            steps.setdefault(5, []).append(
                                lambda i2=i: emit_qk_tile(1, i2))
                            steps.setdefault(13, []).append(
                                lambda i2=i: emit_qk_tile(3, i2))
                        if pend is not None:
                            pp_, ii_, pvA_, pvB_ = pend
                            box = []
                            bcbox = []
                            at_start.append(
                                lambda a=pvA_, b=pvB_, bx=box: bx.extend(
                                    norm_evict(a, b)))
                            rbox = []
                            steps.setdefault(1, []).append(
                                lambda bx=box, rb=rbox: rb.extend(
                                    norm_recip(bx)))
                            steps.setdefault(2, []).append(
                                lambda rb=rbox, bb=bcbox: bb.extend(
                                    norm_bcmm(rb)))
                            steps.setdefault(6, []).append(
                                lambda p2=pp_, i2=ii_, bx=box, bb=bcbox:
                                    norm_stt(p2, i2, bx, bb))
                            if pp_ == 1 and phase == "full":
                                # out-projection for ii_ right after its stt
                                for k, si in enumerate((8, 10, 12, 14)):
                                    steps.setdefault(si, []).append(
                                        lambda i2=ii_, k2=k: emit_outproj(
                                            i2, range(k2, k2 + 1)))
                        with prio():
                            pvA, pvB = emit_attn(p, i, at_start, steps)
                            attn_base[0] = tc.cur_priority
                        pend = (p, i, pvA, pvB)
                    # tail: last chunk's normalize + final out-projections
                    with prio():
                        pp_, ii_, pvA_, pvB_ = pend
                        raws = norm_evict(pvA_, pvB_)
                        norm_rest(pp_, ii_, raws)
                        if phase == "full":
                            emit_outproj(ii_)
                        attn_base[0] = tc.cur_priority

                if reps == 1:
                    emit_body()
                else:
                    with tc.For_i(0, reps, 1):
                        emit_body()
                if phase == "proj":
                    nc.sync.dma_start(out=dbg[:, 0:640],
                                      in_=q8[0][:].bitcast(F32))
                    nc.sync.dma_start(out=dbg[:, 640:1280],
                                      in_=q8[1][:].bitcast(F32))
                    nc.sync.dma_start(out=dbg[:, 1280:2304],
                                      in_=k8[0][:].bitcast(F32))
                    nc.sync.dma_start(out=dbg[:, 2304:3328],
                                      in_=k8[1][:].bitcast(F32))
                    for m in range(NJ // 2):
                        w = HPC * 2 * VSTRIDE // 4
                        nc.sync.dma_start(
                            out=dbg[:, 3328 + m * w:3328 + (m + 1) * w],
                            in_=v8[m][:].bitcast(F32))
                elif phase == "noout":
                    for k in range(2):
                        nc.sync.dma_start(
                            out=dbg[:, k * 2048:(k + 1) * 2048],
                            in_=valsT[k][:].bitcast(F32))

    _split_excess_waits(nc)
    return nc


_program_cache = None


def _get_program():
    global _program_cache
    if _program_cache is None:
        _program_cache = _build_program()
    return _program_cache


# ---------------------------------------------------------------------------
# Host-side sharding + gather
# ---------------------------------------------------------------------------

def _shard_inputs(x, qkv_w, qkv_b, out_w):
    """Build the 8 per-core input maps."""
    x = np.asarray(x, np.float32)
    qkv_w = np.asarray(qkv_w, np.float32)
    qkv_b = np.asarray(qkv_b, np.float32)
    out_w = np.asarray(out_w, np.float32)

    # per-head q/k/v rows of the fused projection: head h covers rows
    # [h*3*HD, (h+1)*3*HD) split q | k | v
    qw = np.stack([qkv_w[h * 3 * HD: h * 3 * HD + HD] for h in range(NH)])
    kw = np.stack([qkv_w[h * 3 * HD + HD: h * 3 * HD + 2 * HD] for h in range(NH)])
    vw = np.stack([qkv_w[h * 3 * HD + 2 * HD: h * 3 * HD + 3 * HD] for h in range(NH)])
    qb = np.stack([qkv_b[h * 3 * HD: h * 3 * HD + HD] for h in range(NH)])
    kb = np.stack([qkv_b[h * 3 * HD + HD: h * 3 * HD + 2 * HD] for h in range(NH)])
    vb = np.stack([qkv_b[h * 3 * HD + 2 * HD: h * 3 * HD + 3 * HD] for h in range(NH)])

    xT = [np.ascontiguousarray(x[b].T) for b in range(B)]

    in_maps = []
    for c in range(NCORES):
        b = c // HPC
        g = c % HPC
        hs = slice(g * HPC, (g + 1) * HPC)
        # [4, HD, D] -> [4*HD, D]; NO softmax scale folding (exp applies it)
        Wq = qw[hs].reshape(HPC * HD, D)
        Wk = kw[hs].reshape(HPC * HD, D)
        Wv = vw[hs].reshape(HPC * HD, D)
        bq = qb[hs].reshape(HPC * HD)
        bk = kb[hs].reshape(HPC * HD)
        bv = vb[hs].reshape(HPC * HD)

        wqk_c = np.ascontiguousarray(np.concatenate([Wq, Wk], 0).T)  # [D, 512]
        bqk_full = np.concatenate([bq, bk])                          # [512]
        bqk_c = np.ascontiguousarray(bqk_full.reshape(-1, 128).T)    # [128, 4]
        wv_c = np.ascontiguousarray(Wv.T)                            # [D, 256]
        bvr_c = np.ascontiguousarray(np.broadcast_to(bv, (128, HPC * HD)))
        # out_w columns for these heads, transposed: [256, D]
        cols = np.arange(g * HPC * HD, (g + 1) * HPC * HD)
        ot_c = np.ascontiguousarray(out_w[:, cols].T)

        in_maps.append({
            "xT": xT[b],
            "wqk": wqk_c,
            "bqk": bqk_c,
            "wv": wv_c,
            "bvr": bvr_c,
            "ot": ot_c,
        })
    return in_maps


def kernel(x, qkv_w, qkv_b, out_w, out_b):
    nc = _get_program()
    in_maps = _shard_inputs(x, qkv_w, qkv_b, out_w)
    res = run_bass_kernel_spmd(nc, in_maps, core_ids=list(range(NCORES)))
    parts = [res.results[c]["out_p"] for c in range(NCORES)]
    out_b = np.asarray(out_b, np.float32)
    out = np.empty((B, S, D), np.float32)
    for b in range(B):
        acc = np.zeros((S, D), np.float64)
        for g in range(HPC):
            acc += np.asarray(parts[b * HPC + g], np.float32)
        out[b] = (acc + out_b[None, :]).astype(np.float32)
    return out
